# revision 41
# baseline (speedup 1.0000x reference)
"""ArcFace (AngularPenaltySMLoss) fused loss kernel for 8 Trainium2 NeuronCores.

Strategy: data-parallel over rows N (each core owns N/8 = 1024 rows of x,
streams the full W).  ~1.5x faster than the bf16 2-slot-psum baseline
(150us -> ~98-100us measured; device power-throttling adds +-3us run noise).

  1. fp8(e4m3) DoubleRow matmul: host uploads W^T [128, 2, 10240] fp8 (scaled
     by SB, zero-padded 10000->10240) and x_n^T [128, 2, 1024] fp8 (row-
     normalized, scaled by SA; SA*SB = S = 30).  Each matmul contracts the
     full K=256 (two 128-planes) in one instruction streaming 1024 rhs rows
     at 2/cycle; 2 matmuls fill one PSUM slot.  The 240 zero-pad classes
     contribute exactly exp(0)=1 each on the ACT-drained last pair; the tail
     subtracts the constant 240.
  2. j-outer main loop over a manually phased 4-slot PSUM ring
     (pm[128, 4, 1024] f32 = all 16KB), slot = global_chunk % 4, so
     consecutive drain PAIRS alternate between slot pairs (0,1)/(2,3) and the
     two drain engines overlap each other and the PE refills.  Drain kinds
     per PAIR_PLAN: 'A' = one ACT exp over [128, 2048] with fused row-sum
     accum_out; 'V' = DVE Schraudolph bit-trick exp (int32(A*v+B) = f32 bit
     pattern of ~exp(v), B tuned for ~zero exp-weighted mean error) + DVE
     reduce of the bitcast; 'H' = hybrid (ACT first chunk, DVE second).
     V/H reduces are deferred one pair so the psum-freeing ts never queues
     behind a reduce.  ~55:25 ACT:DVE chunk split balances the engines.
  3. ACT runs ONLY Exp (one act-table load total): 1/||x|| style math needs
     no ACT at all -- sqrt(S^2-t^2) uses the Quake rsqrt bit trick + 2 Newton
     steps on DVE; exp(numer)/exp(t_s) use the Schraudolph value directly;
     ln(denom) uses a bits->log2 linear + quadratic-frac DVE approximation
     (abs err ~6e-3 on L, ~3e-4 relative on the loss after averaging).
  4. Target path: host pre-gathers (W*SB)[target] as bf16 (data movement
     only); the on-device dot x_n.Wg runs on DVE with fused accumulation;
     numerator = cosM*t_s - sinM*sqrt(S^2-t_s^2) on scaled t_s = S*t.
  5. W streams over two parallel HWDGE queues (SP + Activation) so the
     j-outer sweep is never starved; x^T lands first for a ~13us pipeline
     start.  Per-core partial sum of L_i; host combines 8 scalars.
"""

import math

import numpy as np

S = 30.0
MARGIN = 0.3
EPS = 1e-7
N, D, C = 8192, 256, 10000
NCORES = 8
NLOC = N // NCORES  # 1024 rows per core
NJ = NLOC // 128  # 8 row-chunks of 128 partitions
CP = 10240  # padded class count
CW = 1024  # class-chunk width (one PSUM slot)
NCH = CP // CW  # 10 chunks per row-block
NPAIR = NCH // 2  # 5 drain pairs per row-block
NPAD = CP - C  # 240 zero-pad classes -> exp contributes exactly NPAD
SA = 8.0  # fp8 scale folded into normalized x
SB = 3.75  # fp8 scale folded into W  (SA*SB = S)

# Schraudolph exp constants (f32 domain, int32 bit pattern), B tuned for
# zero exp-weighted mean error: B = 127*2^23 - round(0.0562*2^23)
AEXP = 12102203.0
BEXP = 1064881816.0
RSQRT_MAGIC = 1597463007.0  # 0x5f3759df

# Drain-engine plan per (j, pair): 'A' = ACT, 'V' = DVE Schraudolph,
# 'H' = hybrid (first 1024-chunk on ACT, second on DVE).  A uniform mix per
# row-block keeps both drain engines busy every sweep instead of
# alternating the bottleneck.  Last pair of each j must be 'A' or 'H'
# (ACT half owns no pad; pads are in pair 4 = 'A').
PAIR_PLAN = [
    "AVAHA",
    "AVAHA",
    "AVAHA",
    "AVAHA",
    "AVAHA",
    "AVAHA",
    "AVAHA",
    "AVAHA",
]

_CACHE = {}


def _build():
    import concourse.bass as bass  # noqa: F401
    import concourse.mybir as mybir
    import concourse.tile as tile
    from concourse import bacc

    f32 = mybir.dt.float32
    bf16 = mybir.dt.bfloat16
    f8 = mybir.dt.float8e4
    i32 = mybir.dt.int32
    AF = mybir.ActivationFunctionType
    OP = mybir.AluOpType
    DR = mybir.MatmulPerfMode.DoubleRow

    nc = bacc.Bacc()
    xT_ext = nc.declare_dram_parameter("xT", [128, 2, NLOC], f8, isOutput=False)
    wt_ext = nc.declare_dram_parameter("wt", [128, 2, CP], f8, isOutput=False)
    xnb_ext = nc.declare_dram_parameter("xnb", [128, NJ, D], bf16, isOutput=False)
    wg_ext = nc.declare_dram_parameter("wg", [128, NJ, D], bf16, isOutput=False)
    out_ext = nc.declare_dram_parameter("out", [1, 1], f32, isOutput=True)

    with tile.TileContext(nc) as tc:
        with (
            tc.tile_pool(name="singles", bufs=1) as singles,
            tc.tile_pool(name="idpool", bufs=2) as idpool,
            tc.tile_pool(name="pmain", bufs=1, space="PSUM") as psum_main,
        ):
            # the whole PSUM as a manually phased 4-slot ring
            pm = psum_main.tile([128, 4, CW], f32)

            # ---------------- loads (j=0 critical path first) ------------
            xT = singles.tile([128, 2, NLOC], f8)
            wt = singles.tile([128, 2, CP], f8)
            xnb = singles.tile([128, NJ, D], bf16)
            wg = singles.tile([128, NJ, D], bf16)
            # two parallel HWDGE queues: SP streams all W rounds in sweep
            # order; the Activation queue carries x^T + the target-path pair
            nc.scalar.dma_start(out=xT, in_=xT_ext[:, :, :])
            for r in range(NPAIR):
                c0 = r * 2048
                eng = nc.sync if r % 2 == 0 else nc.scalar
                eng.dma_start(
                    out=wt[:, :, c0 : c0 + 2048], in_=wt_ext[:, :, c0 : c0 + 2048]
                )
            nc.sync.dma_start(out=xnb, in_=xnb_ext[:, :, :])
            nc.sync.dma_start(out=wg, in_=wg_ext[:, :, :])

            # ---------------- target-score path (DVE, off critical path) --
            traw = singles.tile([128, NJ], f32)
            tprod = singles.tile([128, D], bf16)

            def tdot(j):
                nc.vector.scalar_tensor_tensor(
                    out=tprod,
                    in0=xnb[:, j, :],
                    scalar=1.0,
                    in1=wg[:, j, :],
                    op0=OP.mult,
                    op1=OP.mult,
                    accum_out=traw[:, j : j + 1],
                )

            rs_seed = singles.tile([128, NJ], i32)
            rs_t1 = singles.tile([128, NJ], f32)
            rs_y1 = singles.tile([128, NJ], f32)
            rs_t2 = singles.tile([128, NJ], f32)

            def rsqrt2(src, dst, fold=1.0):
                # Quake rsqrt + 2 Newton iterations; dst = fold/sqrt(src)
                nc.vector.tensor_scalar(
                    out=rs_seed,
                    in0=src.bitcast(i32),
                    scalar1=-0.5,
                    scalar2=RSQRT_MAGIC,
                    op0=OP.mult,
                    op1=OP.add,
                )
                y0 = rs_seed.bitcast(f32)
                nc.vector.tensor_tensor(out=rs_t1, in0=y0, in1=y0, op=OP.mult)
                nc.vector.tensor_tensor(out=rs_t1, in0=rs_t1, in1=src, op=OP.mult)
                nc.vector.tensor_scalar(
                    out=rs_t1, in0=rs_t1, scalar1=-0.5, scalar2=1.5,
                    op0=OP.mult, op1=OP.add,
                )
                nc.vector.tensor_tensor(out=rs_y1, in0=y0, in1=rs_t1, op=OP.mult)
                nc.vector.tensor_tensor(out=rs_t2, in0=rs_y1, in1=rs_y1, op=OP.mult)
                nc.vector.tensor_tensor(out=rs_t2, in0=rs_t2, in1=src, op=OP.mult)
                nc.vector.tensor_scalar(
                    out=rs_t2, in0=rs_t2, scalar1=-0.5 * fold, scalar2=1.5 * fold,
                    op0=OP.mult, op1=OP.add,
                )
                nc.vector.tensor_tensor(out=dst, in0=rs_y1, in1=rs_t2, op=OP.mult)

            def numer_chain():
                sclip = S * (1.0 - EPS)
                nc.vector.tensor_scalar(
                    out=tcl, in0=traw, scalar1=-sclip, scalar2=sclip,
                    op0=OP.max, op1=OP.min,
                )
                nc.vector.tensor_tensor(out=usq, in0=tcl, in1=tcl, op=OP.mult)
                nc.vector.tensor_scalar(
                    out=usq, in0=usq, scalar1=-1.0, scalar2=S * S,
                    op0=OP.mult, op1=OP.add,
                )
                # rtm = -sinM*sqrt(usq) = usq * (-sinM * rsqrt(usq))
                rsqrt2(usq, rsu, fold=-math.sin(MARGIN))
                nc.vector.tensor_tensor(out=rtm, in0=usq, in1=rsu, op=OP.mult)
                nc.vector.scalar_tensor_tensor(
                    out=numer, in0=tcl, scalar=math.cos(MARGIN), in1=rtm,
                    op0=OP.mult, op1=OP.add,
                )

            tcl = singles.tile([128, NJ], f32)
            usq = singles.tile([128, NJ], f32)
            rsu = singles.tile([128, NJ], f32)
            rtm = singles.tile([128, NJ], f32)
            numer = singles.tile([128, NJ], f32)
            exp_num = singles.tile([128, NJ], f32)
            exp_st = singles.tile([128, NJ], f32)

            # ---------------- main loop: j outer, chunk pairs inner --------
            # acc columns NPAIR.. hold the hybrid pairs' DVE halves
            acc = singles.tile([128, NJ, NPAIR + 2], f32)
            nc.gpsimd.memset(acc, 0.0)
            edump = singles.tile([128, 2 * CW], bf16)

            pending_red = []

            def flush_red():
                while pending_red:
                    in_ap, accslot = pending_red.pop(0)
                    nc.vector.tensor_reduce(
                        out=accslot,
                        in_=in_ap,
                        axis=mybir.AxisListType.X,
                        op=OP.add,
                    )

            def sch_chunk(src, accslot):
                # DVE Schraudolph into int32; the (bitcast) reduce is deferred
                # until after the NEXT pair's ts so psum slots free sooner
                idump = idpool.tile([128, 2 * CW], i32, tag="id")
                nelem = src.free_size()
                nc.vector.tensor_scalar(
                    out=idump[:, :nelem],
                    in0=src,
                    scalar1=AEXP,
                    scalar2=BEXP,
                    op0=OP.mult,
                    op1=OP.add,
                )
                deferred = (idump[:, :nelem].bitcast(f32), accslot)
                flush_red()
                pending_red.append(deferred)

            def drain_pair(j, p, slot0, hidx):
                kind = PAIR_PLAN[j][p]
                if j == 0 and p == 0 and kind == "A":
                    # stream warm-up: two 1024-wide drains, the first gated
                    # only on chunk 0's matmuls
                    nc.scalar.activation(
                        out=edump[:, :CW],
                        in_=pm[:, slot0, :],
                        func=AF.Exp,
                        accum_out=acc[:, j, p : p + 1],
                    )
                    nc.scalar.activation(
                        out=edump[:, CW:],
                        in_=pm[:, slot0 + 1, :],
                        func=AF.Exp,
                        accum_out=acc[:, j, NPAIR + 1 : NPAIR + 2],
                    )
                elif kind == "A":
                    nc.scalar.activation(
                        out=edump,
                        in_=pm[:, slot0 : slot0 + 2, :],
                        func=AF.Exp,
                        accum_out=acc[:, j, p : p + 1],
                    )
                elif kind == "V":
                    sch_chunk(pm[:, slot0 : slot0 + 2, :], acc[:, j, p : p + 1])
                else:  # hybrid: ACT takes the first slot, DVE the second
                    nc.scalar.activation(
                        out=edump[:, :CW],
                        in_=pm[:, slot0, :],
                        func=AF.Exp,
                        accum_out=acc[:, j, p : p + 1],
                    )
                    sch_chunk(
                        pm[:, slot0 + 1, :], acc[:, j, NPAIR + hidx : NPAIR + hidx + 1]
                    )

            rowsum = singles.tile([128, NJ], f32)
            g = 0  # global chunk counter -> PSUM slot phase
            for j in range(NJ):
                hidx = 0
                for c in range(NCH):
                    for s_ in range(2):
                        nc.tensor.matmul(
                            out=pm[:, g % 4, s_ * 512 : (s_ + 1) * 512],
                            lhsT=xT[:, :, j * 128 : (j + 1) * 128],
                            rhs=wt[:, :, c * CW + s_ * 512 : c * CW + (s_ + 1) * 512],
                            start=True,
                            stop=True,
                            perf_mode=DR,
                            skip_group_check=True,
                        )
                    g += 1
                    if c % 2 == 1:
                        p = c // 2
                        drain_pair(j, p, (g - 2) % 4, hidx)
                        if PAIR_PLAN[j][p] == "H":
                            hidx += 1
                if j == 0:
                    # DVE target-path work slots in behind the first sweep
                    for jj in range(NJ):
                        tdot(jj)
                    numer_chain()
                elif j == 1:
                    nc.vector.tensor_scalar(
                        out=exp_num.bitcast(i32), in0=numer, scalar1=AEXP,
                        scalar2=BEXP, op0=OP.mult, op1=OP.add,
                    )
                    nc.vector.tensor_scalar(
                        out=exp_st.bitcast(i32), in0=tcl, scalar1=AEXP,
                        scalar2=BEXP, op0=OP.mult, op1=OP.add,
                    )

            flush_red()

            # ---------------- combine ----------------
            dnum = singles.tile([128, NJ], f32)  # exp(numer) - exp(t_s)
            nc.vector.tensor_tensor(out=dnum, in0=exp_num, in1=exp_st, op=OP.subtract)
            nc.vector.tensor_reduce(
                out=rowsum, in_=acc, axis=mybir.AxisListType.X, op=OP.add
            )
            denom = singles.tile([128, NJ], f32)
            nc.vector.scalar_tensor_tensor(
                out=denom,
                in0=rowsum,
                scalar=-float(NPAD),
                in1=dnum,
                op0=OP.add,
                op1=OP.add,
            )
            # ln(denom) on DVE: y = bits/2^23 - 127 = e + m;
            # ln(d) ~= ln2*(y + K2*m*(1-m)) with m = frac(y), robust to the
            # f32->int convert being either trunc or round-to-nearest.
            K2 = 0.3398
            ly = singles.tile([128, NJ], f32)
            nc.vector.tensor_scalar(
                out=ly, in0=denom.bitcast(i32), scalar1=1.0 / (1 << 23),
                scalar2=-127.0, op0=OP.mult, op1=OP.add,
            )
            lyi = singles.tile([128, NJ], i32)
            nc.vector.tensor_scalar(
                out=lyi, in0=ly, scalar1=1.0, scalar2=None, op0=OP.mult
            )
            lm0 = singles.tile([128, NJ], f32)
            nc.vector.tensor_tensor(out=lm0, in0=ly, in1=lyi, op=OP.subtract)
            lneg = singles.tile([128, NJ], f32)
            nc.vector.tensor_scalar(
                out=lneg, in0=lm0, scalar1=0.0, scalar2=None, op0=OP.is_lt
            )
            lm = singles.tile([128, NJ], f32)
            nc.vector.tensor_tensor(out=lm, in0=lm0, in1=lneg, op=OP.add)
            lom = singles.tile([128, NJ], f32)
            nc.vector.tensor_scalar(
                out=lom, in0=lm, scalar1=-1.0, scalar2=1.0, op0=OP.mult, op1=OP.add
            )
            lq = singles.tile([128, NJ], f32)
            nc.vector.tensor_tensor(out=lq, in0=lm, in1=lom, op=OP.mult)
            la = singles.tile([128, NJ], f32)
            nc.vector.scalar_tensor_tensor(
                out=la, in0=lq, scalar=K2, in1=ly, op0=OP.mult, op1=OP.add
            )
            Lt = singles.tile([128, NJ], f32)
            nc.vector.scalar_tensor_tensor(
                out=Lt, in0=la, scalar=-math.log(2.0), in1=numer,
                op0=OP.mult, op1=OP.add,
            )
            Lrow = singles.tile([128, 1], f32)
            nc.vector.tensor_reduce(
                out=Lrow, in_=Lt, axis=mybir.AxisListType.X, op=OP.add
            )
            ones = singles.tile([128, 1], f32)
            nc.vector.memset(ones, 1.0)
            nc.tensor.matmul(
                out=pm[0:1, 3, 0:1], lhsT=Lrow, rhs=ones, start=True, stop=True
            )
            Lp = singles.tile([1, 1], f32)
            nc.vector.tensor_copy(out=Lp, in_=pm[0:1, 3, 0:1])
            nc.sync.dma_start(out=out_ext[:, :], in_=Lp)

    nc.finalize()
    return nc


def _get_nc():
    if "nc" not in _CACHE:
        _CACHE["nc"] = _build()
    return _CACHE["nc"]


def prepare_in_maps(x, W, target):
    import ml_dtypes

    f8 = ml_dtypes.float8_e4m3fn
    bf = ml_dtypes.bfloat16

    x = np.asarray(x, dtype=np.float32)
    W = np.asarray(W, dtype=np.float32)
    tgt = np.asarray(target).astype(np.int64).reshape(N)

    xn = x / np.linalg.norm(x, axis=1, keepdims=True)
    xna = (xn * np.float32(SA)).astype(np.float32)

    ws = W * np.float32(SB)
    # W^T in [partition(=d%128), plane(=d//128), class] fp8 layout, zero-padded
    wt = np.zeros((128, 2, CP), dtype=f8)
    wt[:, :, :C] = ws.T.reshape(2, 128, C).transpose(1, 0, 2).astype(f8)
    wgather = ws[tgt].astype(bf)  # [N, D] bf16

    in_maps = []
    for c in range(NCORES):
        sl = slice(c * NLOC, (c + 1) * NLOC)
        xs, wgs = xna[sl], wgather[sl]
        in_maps.append(
            {
                # x_n^T fp8 [d%128, d//128, row]
                "xT": np.ascontiguousarray(
                    xs.T.reshape(2, 128, NLOC).transpose(1, 0, 2).astype(f8)
                ),
                "wt": wt,
                # x_n bf16 [row%128, row//128, d] (for the target dot)
                "xnb": np.ascontiguousarray(
                    xs.reshape(NJ, 128, D).transpose(1, 0, 2).astype(bf)
                ),
                "wg": np.ascontiguousarray(wgs.reshape(NJ, 128, D).transpose(1, 0, 2)),
            }
        )
    return in_maps


def kernel(x, W, target):
    from concourse.bass_utils import run_bass_kernel_spmd

    nc = _get_nc()
    in_maps = prepare_in_maps(x, W, target)
    res = run_bass_kernel_spmd(nc, in_maps, core_ids=list(range(NCORES)))
    parts = np.stack(
        [res.results[i]["out"].astype(np.float32).reshape(()) for i in range(NCORES)]
    )
    total = np.sum(parts, dtype=np.float32)
    return np.float32(-(total / np.float32(N)))


# revision 42
# speedup vs baseline: 1.1192x; 1.1192x over previous
"""ArcFace (AngularPenaltySMLoss) fused loss kernel for 8 Trainium2 NeuronCores.

Strategy: data-parallel over rows N (each core owns N/8 = 1024 rows of x,
streams the full W).  ~1.5x faster than the bf16 2-slot-psum baseline
(150us -> ~98-100us measured; device power-throttling adds +-3us run noise).

  1. fp8(e4m3) DoubleRow matmul: host uploads W^T [128, 2, 10240] fp8 (scaled
     by SB, zero-padded 10000->10240) and x_n^T [128, 2, 1024] fp8 (row-
     normalized, scaled by SA; SA*SB = S = 30).  Each matmul contracts the
     full K=256 (two 128-planes) in one instruction streaming 1024 rhs rows
     at 2/cycle; 2 matmuls fill one PSUM slot.  The 240 zero-pad classes
     contribute exactly exp(0)=1 each on the ACT-drained last pair; the tail
     subtracts the constant 240.
  2. j-outer main loop over a manually phased 4-slot PSUM ring
     (pm[128, 4, 1024] f32 = all 16KB), slot = global_chunk % 4, so
     consecutive drain PAIRS alternate between slot pairs (0,1)/(2,3) and the
     two drain engines overlap each other and the PE refills.  Drain kinds
     per PAIR_PLAN: 'A' = one ACT exp over [128, 2048] with fused row-sum
     accum_out; 'V' = DVE Schraudolph bit-trick exp (int32(A*v+B) = f32 bit
     pattern of ~exp(v), B tuned for ~zero exp-weighted mean error) + DVE
     reduce of the bitcast; 'H' = hybrid (ACT first chunk, DVE second).
     V/H reduces are deferred one pair so the psum-freeing ts never queues
     behind a reduce.  ~55:25 ACT:DVE chunk split balances the engines.
  3. ACT runs ONLY Exp (one act-table load total): 1/||x|| style math needs
     no ACT at all -- sqrt(S^2-t^2) uses the Quake rsqrt bit trick + 2 Newton
     steps on DVE; exp(numer)/exp(t_s) use the Schraudolph value directly;
     ln(denom) uses a bits->log2 linear + quadratic-frac DVE approximation
     (abs err ~6e-3 on L, ~3e-4 relative on the loss after averaging).
  4. Target path: host pre-gathers (W*SB)[target] as bf16 (data movement
     only); the on-device dot x_n.Wg runs on DVE with fused accumulation;
     numerator = cosM*t_s - sinM*sqrt(S^2-t_s^2) on scaled t_s = S*t.
  5. W streams over two parallel HWDGE queues (SP + Activation) so the
     j-outer sweep is never starved; x^T lands first for a ~13us pipeline
     start.  Per-core partial sum of L_i; host combines 8 scalars.
"""

import math

import numpy as np

S = 30.0
MARGIN = 0.3
EPS = 1e-7
N, D, C = 8192, 256, 10000
NCORES = 8
NLOC = N // NCORES  # 1024 rows per core
NJ = NLOC // 128  # 8 row-chunks of 128 partitions
CP = 10240  # padded class count
CW = 1024  # class-chunk width (one PSUM slot)
NCH = CP // CW  # 10 chunks per row-block
NPAIR = NCH // 2  # 5 drain pairs per row-block
NPAD = CP - C  # 240 zero-pad classes -> exp contributes exactly NPAD
SA = 8.0  # fp8 scale folded into normalized x
SB = 3.75  # fp8 scale folded into W  (SA*SB = S)

# Schraudolph exp constants (f32 domain, int32 bit pattern), B tuned for
# zero exp-weighted mean error: B = 127*2^23 - round(0.0562*2^23)
AEXP = 12102203.0
BEXP = 1064881816.0
RSQRT_MAGIC = 1597463007.0  # 0x5f3759df

# Drain-engine plan per (j, pair): 'A' = ACT, 'V' = DVE Schraudolph,
# 'H' = hybrid (first 1024-chunk on ACT, second on DVE).  A uniform mix per
# row-block keeps both drain engines busy every sweep instead of
# alternating the bottleneck.  Last pair of each j must be 'A' or 'H'
# (ACT half owns no pad; pads are in pair 4 = 'A').
PAIR_PLAN = [
    "AVAHA",
    "AVAHA",
    "AVAHA",
    "AVHHA",
    "AVAHA",
    "AVAHA",
    "AVAHA",
    "AVAHA",
]

_CACHE = {}


def _build():
    import concourse.bass as bass  # noqa: F401
    import concourse.mybir as mybir
    import concourse.tile as tile
    from concourse import bacc

    f32 = mybir.dt.float32
    bf16 = mybir.dt.bfloat16
    f8 = mybir.dt.float8e4
    i32 = mybir.dt.int32
    AF = mybir.ActivationFunctionType
    OP = mybir.AluOpType
    DR = mybir.MatmulPerfMode.DoubleRow

    nc = bacc.Bacc()
    xT_ext = nc.declare_dram_parameter("xT", [128, 2, NLOC], f8, isOutput=False)
    wt_ext = nc.declare_dram_parameter("wt", [128, 2, CP], f8, isOutput=False)
    xnb_ext = nc.declare_dram_parameter("xnb", [128, NJ, D], bf16, isOutput=False)
    wg_ext = nc.declare_dram_parameter("wg", [128, NJ, D], bf16, isOutput=False)
    out_ext = nc.declare_dram_parameter("out", [1, 1], f32, isOutput=True)

    with tile.TileContext(nc) as tc:
        with (
            tc.tile_pool(name="singles", bufs=1) as singles,
            tc.tile_pool(name="idpool", bufs=2) as idpool,
            tc.tile_pool(name="pmain", bufs=1, space="PSUM") as psum_main,
        ):
            # the whole PSUM as a manually phased 4-slot ring
            pm = psum_main.tile([128, 4, CW], f32)

            # ---------------- loads (j=0 critical path first) ------------
            xT = singles.tile([128, 2, NLOC], f8)
            wt = singles.tile([128, 2, CP], f8)
            xnb = singles.tile([128, NJ, D], bf16)
            wg = singles.tile([128, NJ, D], bf16)
            # two parallel HWDGE queues: SP streams all W rounds in sweep
            # order; the Activation queue carries x^T + the target-path pair
            nc.scalar.dma_start(out=xT, in_=xT_ext[:, :, :])
            for r in range(NPAIR):
                c0 = r * 2048
                eng = nc.sync if r % 2 == 0 else nc.scalar
                eng.dma_start(
                    out=wt[:, :, c0 : c0 + 2048], in_=wt_ext[:, :, c0 : c0 + 2048]
                )
            nc.sync.dma_start(out=xnb, in_=xnb_ext[:, :, :])
            nc.sync.dma_start(out=wg, in_=wg_ext[:, :, :])

            # ---------------- target-score path (DVE, off critical path) --
            traw = singles.tile([128, NJ], f32)
            tprod = singles.tile([128, D], bf16)

            def tdot(j):
                nc.vector.scalar_tensor_tensor(
                    out=tprod,
                    in0=xnb[:, j, :],
                    scalar=1.0,
                    in1=wg[:, j, :],
                    op0=OP.mult,
                    op1=OP.mult,
                    accum_out=traw[:, j : j + 1],
                )

            rs_seed = singles.tile([128, NJ], i32)
            rs_t1 = singles.tile([128, NJ], f32)
            rs_y1 = singles.tile([128, NJ], f32)
            rs_t2 = singles.tile([128, NJ], f32)

            def rsqrt2(src, dst, fold=1.0):
                # Quake rsqrt + 2 Newton iterations; dst = fold/sqrt(src)
                nc.vector.tensor_scalar(
                    out=rs_seed,
                    in0=src.bitcast(i32),
                    scalar1=-0.5,
                    scalar2=RSQRT_MAGIC,
                    op0=OP.mult,
                    op1=OP.add,
                )
                y0 = rs_seed.bitcast(f32)
                nc.vector.tensor_tensor(out=rs_t1, in0=y0, in1=y0, op=OP.mult)
                nc.vector.tensor_tensor(out=rs_t1, in0=rs_t1, in1=src, op=OP.mult)
                nc.vector.tensor_scalar(
                    out=rs_t1, in0=rs_t1, scalar1=-0.5, scalar2=1.5,
                    op0=OP.mult, op1=OP.add,
                )
                nc.vector.tensor_tensor(out=rs_y1, in0=y0, in1=rs_t1, op=OP.mult)
                nc.vector.tensor_tensor(out=rs_t2, in0=rs_y1, in1=rs_y1, op=OP.mult)
                nc.vector.tensor_tensor(out=rs_t2, in0=rs_t2, in1=src, op=OP.mult)
                nc.vector.tensor_scalar(
                    out=rs_t2, in0=rs_t2, scalar1=-0.5 * fold, scalar2=1.5 * fold,
                    op0=OP.mult, op1=OP.add,
                )
                nc.vector.tensor_tensor(out=dst, in0=rs_y1, in1=rs_t2, op=OP.mult)

            def numer_chain():
                sclip = S * (1.0 - EPS)
                nc.vector.tensor_scalar(
                    out=tcl, in0=traw, scalar1=-sclip, scalar2=sclip,
                    op0=OP.max, op1=OP.min,
                )
                nc.vector.tensor_tensor(out=usq, in0=tcl, in1=tcl, op=OP.mult)
                nc.vector.tensor_scalar(
                    out=usq, in0=usq, scalar1=-1.0, scalar2=S * S,
                    op0=OP.mult, op1=OP.add,
                )
                # rtm = -sinM*sqrt(usq) = usq * (-sinM * rsqrt(usq))
                rsqrt2(usq, rsu, fold=-math.sin(MARGIN))
                nc.vector.tensor_tensor(out=rtm, in0=usq, in1=rsu, op=OP.mult)
                nc.vector.scalar_tensor_tensor(
                    out=numer, in0=tcl, scalar=math.cos(MARGIN), in1=rtm,
                    op0=OP.mult, op1=OP.add,
                )

            tcl = singles.tile([128, NJ], f32)
            usq = singles.tile([128, NJ], f32)
            rsu = singles.tile([128, NJ], f32)
            rtm = singles.tile([128, NJ], f32)
            numer = singles.tile([128, NJ], f32)
            exp_num = singles.tile([128, NJ], f32)
            exp_st = singles.tile([128, NJ], f32)

            # ---------------- main loop: j outer, chunk pairs inner --------
            # acc columns NPAIR.. hold the hybrid pairs' DVE halves
            acc = singles.tile([128, NJ, NPAIR + 2], f32)
            nc.gpsimd.memset(acc, 0.0)
            edump = singles.tile([128, 2 * CW], bf16)

            pending_red = []

            def flush_red():
                while pending_red:
                    in_ap, accslot = pending_red.pop(0)
                    nc.vector.tensor_reduce(
                        out=accslot,
                        in_=in_ap,
                        axis=mybir.AxisListType.X,
                        op=OP.add,
                    )

            def sch_chunk(src, accslot):
                # DVE Schraudolph into int32; the (bitcast) reduce is deferred
                # until after the NEXT pair's ts so psum slots free sooner
                idump = idpool.tile([128, 2 * CW], i32, tag="id")
                nelem = src.free_size()
                nc.vector.tensor_scalar(
                    out=idump[:, :nelem],
                    in0=src,
                    scalar1=AEXP,
                    scalar2=BEXP,
                    op0=OP.mult,
                    op1=OP.add,
                )
                deferred = (idump[:, :nelem].bitcast(f32), accslot)
                flush_red()
                pending_red.append(deferred)

            def drain_pair(j, p, slot0, hidx):
                kind = PAIR_PLAN[j][p]
                if kind == "A":
                    nc.scalar.activation(
                        out=edump,
                        in_=pm[:, slot0 : slot0 + 2, :],
                        func=AF.Exp,
                        accum_out=acc[:, j, p : p + 1],
                    )
                elif kind == "V":
                    sch_chunk(pm[:, slot0 : slot0 + 2, :], acc[:, j, p : p + 1])
                else:  # hybrid: ACT takes the first slot, DVE the second
                    nc.scalar.activation(
                        out=edump[:, :CW],
                        in_=pm[:, slot0, :],
                        func=AF.Exp,
                        accum_out=acc[:, j, p : p + 1],
                    )
                    sch_chunk(
                        pm[:, slot0 + 1, :], acc[:, j, NPAIR + hidx : NPAIR + hidx + 1]
                    )

            rowsum = singles.tile([128, NJ], f32)
            g = 0  # global chunk counter -> PSUM slot phase
            for j in range(NJ):
                hidx = 0
                for c in range(NCH):
                    for s_ in range(2):
                        nc.tensor.matmul(
                            out=pm[:, g % 4, s_ * 512 : (s_ + 1) * 512],
                            lhsT=xT[:, :, j * 128 : (j + 1) * 128],
                            rhs=wt[:, :, c * CW + s_ * 512 : c * CW + (s_ + 1) * 512],
                            start=True,
                            stop=True,
                            perf_mode=DR,
                            skip_group_check=True,
                        )
                    g += 1
                    if c % 2 == 1:
                        p = c // 2
                        drain_pair(j, p, (g - 2) % 4, hidx)
                        if PAIR_PLAN[j][p] == "H":
                            hidx += 1
                if j == 0:
                    # DVE target-path work slots in behind the first sweep
                    for jj in range(NJ):
                        tdot(jj)
                    numer_chain()
                elif j == 1:
                    nc.vector.tensor_scalar(
                        out=exp_num.bitcast(i32), in0=numer, scalar1=AEXP,
                        scalar2=BEXP, op0=OP.mult, op1=OP.add,
                    )
                    nc.vector.tensor_scalar(
                        out=exp_st.bitcast(i32), in0=tcl, scalar1=AEXP,
                        scalar2=BEXP, op0=OP.mult, op1=OP.add,
                    )

            flush_red()

            # ---------------- combine ----------------
            dnum = singles.tile([128, NJ], f32)  # exp(numer) - exp(t_s)
            nc.vector.tensor_tensor(out=dnum, in0=exp_num, in1=exp_st, op=OP.subtract)
            nc.vector.tensor_reduce(
                out=rowsum, in_=acc, axis=mybir.AxisListType.X, op=OP.add
            )
            denom = singles.tile([128, NJ], f32)
            nc.vector.scalar_tensor_tensor(
                out=denom,
                in0=rowsum,
                scalar=-float(NPAD),
                in1=dnum,
                op0=OP.add,
                op1=OP.add,
            )
            # ln(denom) on DVE: y = bits/2^23 - 127 = e + m;
            # ln(d) ~= ln2*(y + K2*m*(1-m)) with m = frac(y), robust to the
            # f32->int convert being either trunc or round-to-nearest.
            K2 = 0.3398
            ly = singles.tile([128, NJ], f32)
            nc.vector.tensor_scalar(
                out=ly, in0=denom.bitcast(i32), scalar1=1.0 / (1 << 23),
                scalar2=-127.0, op0=OP.mult, op1=OP.add,
            )
            lyi = singles.tile([128, NJ], i32)
            nc.vector.tensor_scalar(
                out=lyi, in0=ly, scalar1=1.0, scalar2=None, op0=OP.mult
            )
            lm0 = singles.tile([128, NJ], f32)
            nc.vector.tensor_tensor(out=lm0, in0=ly, in1=lyi, op=OP.subtract)
            lneg = singles.tile([128, NJ], f32)
            nc.vector.tensor_scalar(
                out=lneg, in0=lm0, scalar1=0.0, scalar2=None, op0=OP.is_lt
            )
            lm = singles.tile([128, NJ], f32)
            nc.vector.tensor_tensor(out=lm, in0=lm0, in1=lneg, op=OP.add)
            lom = singles.tile([128, NJ], f32)
            nc.vector.tensor_scalar(
                out=lom, in0=lm, scalar1=-1.0, scalar2=1.0, op0=OP.mult, op1=OP.add
            )
            lq = singles.tile([128, NJ], f32)
            nc.vector.tensor_tensor(out=lq, in0=lm, in1=lom, op=OP.mult)
            la = singles.tile([128, NJ], f32)
            nc.vector.scalar_tensor_tensor(
                out=la, in0=lq, scalar=K2, in1=ly, op0=OP.mult, op1=OP.add
            )
            Lt = singles.tile([128, NJ], f32)
            nc.vector.scalar_tensor_tensor(
                out=Lt, in0=la, scalar=-math.log(2.0), in1=numer,
                op0=OP.mult, op1=OP.add,
            )
            Lrow = singles.tile([128, 1], f32)
            nc.vector.tensor_reduce(
                out=Lrow, in_=Lt, axis=mybir.AxisListType.X, op=OP.add
            )
            ones = singles.tile([128, 1], f32)
            nc.vector.memset(ones, 1.0)
            nc.tensor.matmul(
                out=pm[0:1, 3, 0:1], lhsT=Lrow, rhs=ones, start=True, stop=True
            )
            Lp = singles.tile([1, 1], f32)
            nc.vector.tensor_copy(out=Lp, in_=pm[0:1, 3, 0:1])
            nc.sync.dma_start(out=out_ext[:, :], in_=Lp)

    nc.finalize()
    return nc


def _get_nc():
    if "nc" not in _CACHE:
        _CACHE["nc"] = _build()
    return _CACHE["nc"]


def prepare_in_maps(x, W, target):
    import ml_dtypes

    f8 = ml_dtypes.float8_e4m3fn
    bf = ml_dtypes.bfloat16

    x = np.asarray(x, dtype=np.float32)
    W = np.asarray(W, dtype=np.float32)
    tgt = np.asarray(target).astype(np.int64).reshape(N)

    xn = x / np.linalg.norm(x, axis=1, keepdims=True)
    xna = (xn * np.float32(SA)).astype(np.float32)

    ws = W * np.float32(SB)
    # W^T in [partition(=d%128), plane(=d//128), class] fp8 layout, zero-padded
    wt = np.zeros((128, 2, CP), dtype=f8)
    wt[:, :, :C] = ws.T.reshape(2, 128, C).transpose(1, 0, 2).astype(f8)
    wgather = ws[tgt].astype(bf)  # [N, D] bf16

    in_maps = []
    for c in range(NCORES):
        sl = slice(c * NLOC, (c + 1) * NLOC)
        xs, wgs = xna[sl], wgather[sl]
        in_maps.append(
            {
                # x_n^T fp8 [d%128, d//128, row]
                "xT": np.ascontiguousarray(
                    xs.T.reshape(2, 128, NLOC).transpose(1, 0, 2).astype(f8)
                ),
                "wt": wt,
                # x_n bf16 [row%128, row//128, d] (for the target dot)
                "xnb": np.ascontiguousarray(
                    xs.reshape(NJ, 128, D).transpose(1, 0, 2).astype(bf)
                ),
                "wg": np.ascontiguousarray(wgs.reshape(NJ, 128, D).transpose(1, 0, 2)),
            }
        )
    return in_maps


def kernel(x, W, target):
    from concourse.bass_utils import run_bass_kernel_spmd

    nc = _get_nc()
    in_maps = prepare_in_maps(x, W, target)
    res = run_bass_kernel_spmd(nc, in_maps, core_ids=list(range(NCORES)))
    parts = np.stack(
        [res.results[i]["out"].astype(np.float32).reshape(()) for i in range(NCORES)]
    )
    total = np.sum(parts, dtype=np.float32)
    return np.float32(-(total / np.float32(N)))
